# revision 18
# baseline (speedup 1.0000x reference)
"""Trainium2 Bass kernel for batched cross-attention.

Problem (hardcoded shapes):
  img_embeds:          (8, 4096, 512)  f32
  text_embeds:         (8, 512, 768)   f32
  text_attention_mask: (8, 512)        i32
  Wq (512,512), Wk (512,768), Wv (512,768), Wo (512,512), bo (512,)
  out:                 (8, 4096, 512)  f32

Sharding: data-parallel over batch B=8 -> one batch element per NeuronCore
(8 cores). Weights replicated. No collectives needed.

Host prep: masked-out text tokens are compacted away (attention gives them
zero weight), inputs are cast to bf16 (validated: end-to-end rel err ~4e-3
vs the 2e-2 budget), the key-valid mask is laid out per-partition, and the
output bias bo is added on the host after the gather.

Per-core algorithm (all bf16 matmuls, fp32 PSUM accumulation):
  - weights / text arrive transposed into [contract-dim, free] layouts via
    DMA XBAR transposes (2-byte dtype), so the PE never runs transposes.
  - setup: K^T = Wk^T-matmuls, V stored per-head with HD mask columns
    appended (attend's rows [HD:2HD] then come out as the softmax
    denominator, pre-masked).
  - per 512-query block: x^T loaded straight from HBM via DMA transpose;
    Q^T = Wq^T @ x^T; per head: scores^T into a 3-bank PSUM tile, ONE
    batched exp over all 1536 columns on ACT (scale folded in), attend
    accumulated over j-chunks, reciprocal+normalize on DVE; finally
    Y = attn^T.T @ Wo^T with the PSUM->SBUF eviction on DVE (so the next
    block's exp stream never queues behind it on ACT) and the store DMA'd
    out from SP (bias is added host-side).
"""

import os
from contextlib import ExitStack

import numpy as np
import ml_dtypes

import concourse.bass as bass
import concourse.tile as tile
from concourse import bacc, mybir

F32 = mybir.dt.float32
BF16 = mybir.dt.bfloat16

B, N_IMG, N_TXT = 8, 4096, 512
IMG_DIM, TEXT_DIM, H, HD = 512, 768, 8, 64
SCALE = float((TEXT_DIM // H) ** -0.5)
P = 128
N_CORES = 8
_USE_RECIP_APPROX = os.environ.get("KERNEL_RECIP_APPROX", "1") == "1"

IB = N_IMG // 512  # 8 query blocks of 512
NJ = 384  # compacted key count (3 chunks of 128); falls back to 512 if exceeded


def _build_nc(nj: int = NJ, tiny: bool = False, repeat: int = 1, part: str = "all") -> bass.Bass:
    njc = nj // P
    nc = bacc.Bacc("TRN2", target_bir_lowering=False, debug=False)

    img = nc.dram_tensor("img", [N_IMG, IMG_DIM], BF16, kind="ExternalInput").ap()
    txt = nc.dram_tensor("txt", [nj, TEXT_DIM], BF16, kind="ExternalInput").ap()
    msk = nc.dram_tensor("msk", [P, njc], F32, kind="ExternalInput").ap()
    wq = nc.dram_tensor("wq", [IMG_DIM, IMG_DIM], BF16, kind="ExternalInput").ap()
    wk = nc.dram_tensor("wk", [IMG_DIM, TEXT_DIM], BF16, kind="ExternalInput").ap()
    wv = nc.dram_tensor("wv", [IMG_DIM, TEXT_DIM], BF16, kind="ExternalInput").ap()
    wo = nc.dram_tensor("wo", [IMG_DIM, IMG_DIM], BF16, kind="ExternalInput").ap()
    out = nc.dram_tensor("out", [N_IMG, IMG_DIM], F32, kind="ExternalOutput").ap()

    with tile.TileContext(nc) as tc:
        with ExitStack() as ctx:
            if tiny:
                with tc.tile_pool(name="tp", bufs=1) as tp:
                    tt = tp.tile([P, 512], F32, tag="tt")
                    nc.sync.dma_start(tt, out[:P, :])
                    nc.sync.dma_start(out[:P, :], tt)
            else:
                _body(ctx, tc, img, txt, msk, wq, wk, wv, wo, out, njc, repeat, part)
    nc.compile()
    return nc


def _body(ctx, tc, img, txt, msk, wq, wk, wv, wo, out, njc, repeat=1, part="all"):
    nc = tc.nc
    Exp = mybir.ActivationFunctionType.Exp
    nj = njc * P

    out_r = out.rearrange("(n p) d -> p n d", p=P)

    const = ctx.enter_context(tc.tile_pool(name="const", bufs=1))
    # PSUM: sc slots are njc banks each; pq/py reuse the sc slots.
    sc_bufs = 2 if njc <= 3 else 1
    ps3 = ctx.enter_context(tc.tile_pool(name="ps3", bufs=sc_bufs, space="PSUM"))
    atp = ctx.enter_context(tc.tile_pool(name="atp", bufs=2, space="PSUM"))

    # ---- constants / weights (transposed into [contract-dim, free] layouts)
    WqT = const.tile([P, 4, 512], BF16, tag="WqT")  # [d, qd]
    WoT = const.tile([P, 4, 512], BF16, tag="WoT")  # [c, od]
    WkT = const.tile([P, 6, 512], BF16, tag="WkT")  # [td, kd]
    WvT = const.tile([P, 6, 512], BF16, tag="WvT")  # [td, vd]
    tT = const.tile([P, 6, nj], BF16, tag="tT")     # [td, j]
    KT = const.tile([P, 4, nj], BF16, tag="KT")     # [kd, j]
    Vx = const.tile([P, njc, H, 2 * HD], BF16, tag="Vx")  # [j%, jc, h, vd|mask]
    km = const.tile([P, njc], F32, tag="km")
    ones_f = const.tile([P, HD], F32, tag="ones_f")
    nc.any.memset(ones_f, 1.0)
    nc.sync.dma_start(km, msk)

    for oc in range(4):
        nc.sync.dma_start_transpose(WqT[:, oc, :], wq[:, oc * P : (oc + 1) * P])
        nc.sync.dma_start_transpose(WoT[:, oc, :], wo[:, oc * P : (oc + 1) * P])
    for oc in range(6):
        nc.sync.dma_start_transpose(WkT[:, oc, :], wk[:, oc * P : (oc + 1) * P])
        nc.sync.dma_start_transpose(WvT[:, oc, :], wv[:, oc * P : (oc + 1) * P])
        nc.sync.dma_start_transpose(tT[:, oc, :nj], txt[:, oc * P : (oc + 1) * P])

    # K^T[kd, j] = sum_td WkT[td, kd] * tT[td, j]
    for kc in range(4):
        pkt = ps3.tile([P, njc * 512], F32, tag="sc", name=f"pkt_{kc}")
        for t6 in range(6):
            nc.tensor.matmul(
                pkt[:, :nj],
                WkT[:, t6, kc * P : (kc + 1) * P],
                tT[:, t6, :],
                start=(t6 == 0),
                stop=(t6 == 5),
            )
        nc.vector.tensor_copy(KT[:, kc, :], pkt[:, :nj])

    # V[j, vd] = sum_td tT[td, j] * WvT[td, vd]; per-head columns, mask applied
    for jc in range(njc):
        pv = ps3.tile([P, njc * 512], F32, tag="sc", name=f"pv_{jc}")
        for t6 in range(6):
            nc.tensor.matmul(
                pv[:, :512],
                tT[:, t6, jc * P : (jc + 1) * P],
                WvT[:, t6, :],
                start=(t6 == 0),
                stop=(t6 == 5),
            )
        # mask columns FIRST so the denominator lands in partitions 0-63 of
        # `at` (reciprocal_approx_fast requires partition offset 0 on input)
        nc.vector.tensor_scalar_mul(
            Vx[:, jc, :, HD:],
            pv[:, :512].rearrange("p (h v) -> p h v", h=H),
            km[:, jc : jc + 1],
        )
        nc.vector.tensor_scalar_mul(
            Vx[:, jc, :, :HD],
            ones_f[:, None, :].broadcast_to([P, H, HD]),
            km[:, jc : jc + 1],
        )

    # ---- pipelined pools for the main loop
    xtp = ctx.enter_context(tc.tile_pool(name="xtp", bufs=2))
    qtp = ctx.enter_context(tc.tile_pool(name="qtp", bufs=2))
    exp = ctx.enter_context(tc.tile_pool(name="exw", bufs=4))
    anp = ctx.enter_context(tc.tile_pool(name="anp", bufs=2))
    ysp = ctx.enter_context(tc.tile_pool(name="ysp", bufs=3))
    rcp = ctx.enter_context(tc.tile_pool(name="rcp", bufs=4))

    def emit_O(attn, ib):
        # Y[i, od] = sum_c attn[c, i] * WoT[c, od]   (bias added on host)
        # py lives in the 1-bank "at" slots; emitted one block late so these
        # matmuls fill the PE while the next block's exp stream runs.
        for mc in range(4):
            py = atp.tile([P, 512], F32, tag="at", name=f"py_{mc}")
            for cc in range(4):
                nc.tensor.matmul(
                    py,
                    attn[:, cc, mc * P : (mc + 1) * P],
                    WoT[:, cc, :],
                    start=(cc == 0),
                    stop=(cc == 3),
                )
            y_sb = ysp.tile([P, 512], F32, tag="y")
            # DVE (not ACT): keeps the next block's exp stream from queueing
            # behind the y-copy train on the Activation engine
            nc.vector.tensor_copy(y_sb, py)
            nc.sync.dma_start(out_r[:, ib * 4 + mc, :], y_sb)

    def _main_loop():
      for ib in range(IB):
        # x^T for this 512-query block, straight from HBM via XBAR transpose
        xT = xtp.tile([P, 4, 512], BF16, tag="xT")  # [d, i]
        for dc in range(4):
            nc.sync.dma_start_transpose(
                xT[:, dc, :], img[ib * 512 : (ib + 1) * 512, dc * P : (dc + 1) * P]
            )
        if part == "dma":
            y_sb0 = ysp.tile([P, 512], F32, tag="y")
            nc.vector.tensor_copy(y_sb0, xT[:, 0, :])
            nc.scalar.dma_start(out_r[:, ib * 4, :], y_sb0)
            continue

        # Q^T[qd, i] = sum_d WqT[d, qd] * xT[d, i]
        qt = qtp.tile([P, 4, 512], BF16, tag="qt")  # [qd, i]
        for qc in range(4):
            pq = ps3.tile([P, njc * 512], F32, tag="sc", name=f"pq_{qc}")
            for dc in range(4):
                nc.tensor.matmul(
                    pq[:, :512],
                    WqT[:, dc, qc * P : (qc + 1) * P],
                    xT[:, dc, :],
                    start=(dc == 0),
                    stop=(dc == 3),
                )
            nc.vector.tensor_copy(qt[:, qc, :], pq[:, :512])

        attn = anp.tile([P, 4, 512], BF16, tag="attn")  # [c, i] normalized att^T

        def head_scores(h):
            po = (h % 2) * HD
            hc = h // 2
            qh = qt[po : po + HD, hc, :]  # [64, 512]
            sc = ps3.tile([P, njc * 512], F32, tag="sc", name="sc")
            for jc in range(njc):
                nc.tensor.matmul(
                    sc[:, jc * 512 : (jc + 1) * 512],
                    KT[po : po + HD, hc, jc * P : (jc + 1) * P],
                    qh,
                )
            ex = exp.tile([P, njc, 512], BF16, tag="ex", name="ex")
            # one batched exp over all njc*512 columns (scale folded in)
            nc.scalar.activation(ex, sc.rearrange("p (c f) -> p c f", f=512), Exp, scale=SCALE)
            return ex

        def head_attend(h, ex):
            po = (h % 2) * HD
            hc = h // 2
            at = atp.tile([P, 512], F32, tag="at", name="at")
            for jc in range(njc):
                nc.tensor.matmul(
                    at,
                    Vx[:, jc, h, :],
                    ex[:, jc, :],
                    start=(jc == 0),
                    stop=(jc == njc - 1),
                )
            # rows [0:HD] of `at` are the softmax denominator (replicated);
            # rows [HD:2*HD] are the unnormalized attended values
            rec = rcp.tile([HD, 512], F32, tag="rec")
            if _USE_RECIP_APPROX:
                nc.vector.reciprocal_approx_fast(rec, at[:HD, :])
            else:
                nc.vector.reciprocal(rec, at[:HD, :])
            nc.vector.tensor_mul(attn[po : po + HD, hc, :], at[HD:, :], rec)

        # software pipeline: head h's scores/exp overlap head h-1's attend
        prev = None
        for h in range(H):
            ex = head_scores(h)
            if prev is not None:
                head_attend(prev[0], prev[1])
            prev = (h, ex)
        head_attend(prev[0], prev[1])
        emit_O(attn, ib)

    if repeat == 1:
        _main_loop()
    else:
        with tc.For_i(0, repeat, 1):
            _main_loop()


_RUNNERS = {}


def _get_runner(nj: int = NJ, repeat: int = 1):
    """Build the Bass program once per nj and wrap it in a cached 8-core
    shard_map jit (mirrors bass_utils.run_bass_kernel_spmd's axon path, but
    reusable across calls so repeated executions don't recompile)."""
    key = (nj, repeat)
    if key in _RUNNERS:
        return _RUNNERS[key]

    import jax
    from jax.sharding import Mesh, PartitionSpec
    from jax.experimental.shard_map import shard_map
    from concourse import bass2jax

    nc = _build_nc(nj if nj > 0 else NJ, tiny=(nj <= 0), repeat=repeat)
    bass2jax.install_neuronx_cc_hook()

    partition_name = nc.partition_id_tensor.name if nc.partition_id_tensor else None
    in_names = []
    out_names = []
    out_avals = []
    zero_out_shapes = []
    for alloc in nc.m.functions[0].allocations:
        if not isinstance(alloc, mybir.MemoryLocationSet):
            continue
        name = alloc.memorylocations[0].name
        if alloc.kind == "ExternalInput":
            if name != partition_name:
                in_names.append(name)
        elif alloc.kind == "ExternalOutput":
            shape = tuple(alloc.tensor_shape)
            dtype = mybir.dt.np(alloc.dtype)
            out_names.append(name)
            out_avals.append(jax.core.ShapedArray(shape, dtype))
            zero_out_shapes.append((shape, dtype))
    n_params = len(in_names)
    n_outs = len(out_names)
    all_names = list(in_names) + list(out_names)
    if partition_name is not None:
        all_names.append(partition_name)

    def _bodyfn(*args):
        operands = list(args)
        if partition_name is not None:
            operands.append(bass2jax.partition_id_tensor())
        outs = bass2jax._bass_exec_p.bind(
            *operands,
            out_avals=tuple(out_avals),
            in_names=tuple(all_names),
            out_names=tuple(out_names),
            lowering_input_output_aliases=(),
            sim_require_finite=True,
            sim_require_nnan=True,
            nc=nc,
        )
        return tuple(outs)

    devices = jax.devices()[:N_CORES]
    mesh = Mesh(np.asarray(devices), ("core",))
    donate = tuple(range(n_params, n_params + n_outs))
    sharded = jax.jit(
        shard_map(
            _bodyfn,
            mesh=mesh,
            in_specs=(PartitionSpec("core"),) * (n_params + n_outs),
            out_specs=(PartitionSpec("core"),) * n_outs,
            check_rep=False,
        ),
        donate_argnums=donate,
        keep_unused=True,
    )

    _RUNNERS[key] = (sharded, in_names, out_names, zero_out_shapes, nc,
                    (in_names, out_names, out_avals, zero_out_shapes))
    return _RUNNERS[key]


def _concat_inputs(in_maps, in_names):
    return [
        np.concatenate([np.asarray(m[name]) for m in in_maps], axis=0)
        for name in in_names
    ]


def run_cores(in_maps, nj: int = NJ):
    """Run the SPMD program; in_maps is a list of 8 dicts name->array.
    Returns list of 8 dicts name->array."""
    sharded, in_names, out_names, zero_out_shapes = _get_runner(nj)[:4]
    concat_in = _concat_inputs(in_maps, in_names)
    concat_zeros = [
        np.zeros((N_CORES * s[0],) + tuple(s[1:]), dt) for (s, dt) in zero_out_shapes
    ]
    outs = sharded(*concat_in, *concat_zeros)
    outs = [np.asarray(o) for o in outs]
    per_core = []
    for c in range(N_CORES):
        d = {}
        for i, name in enumerate(out_names):
            shape = zero_out_shapes[i][0]
            d[name] = outs[i].reshape((N_CORES,) + tuple(shape))[c]
        per_core.append(d)
    return per_core


def _make_in_maps(img_embeds, text_embeds, text_attention_mask, Wq, Wk, Wv, Wo, bo):
    """Shard per batch element; compact masked-out text tokens away; cast the
    operands the device consumes to bf16; build the per-partition key mask.

    Attention only reads text tokens with mask != 0 (softmax gives them zero
    weight), so we gather the valid rows and pad to NJ. If some batch has
    more than NJ valid tokens, fall back to the uncompacted 512-key kernel.
    Returns (in_maps, nj)."""
    bf16 = ml_dtypes.bfloat16
    img_embeds = np.asarray(img_embeds, dtype=np.float32)
    text_embeds = np.asarray(text_embeds, dtype=np.float32)
    msk = np.asarray(text_attention_mask, dtype=np.int32)
    Wqb = np.ascontiguousarray(np.asarray(Wq, dtype=np.float32).astype(bf16))
    Wkb = np.ascontiguousarray(np.asarray(Wk, dtype=np.float32).astype(bf16))
    Wvb = np.ascontiguousarray(np.asarray(Wv, dtype=np.float32).astype(bf16))
    Wob = np.ascontiguousarray(np.asarray(Wo, dtype=np.float32).astype(bf16))

    idxs = [np.nonzero(msk[b])[0] for b in range(B)]
    nj = NJ if max(len(ix) for ix in idxs) <= NJ else N_TXT
    njc = nj // P

    in_maps = []
    for b in range(B):
        ix = idxs[b]
        t_c = np.zeros((nj, TEXT_DIM), dtype=bf16)
        t_c[: len(ix)] = text_embeds[b][ix].astype(bf16)
        v = np.zeros((nj,), dtype=np.float32)
        v[: len(ix)] = 1.0
        in_maps.append(
            {
                "img": np.ascontiguousarray(img_embeds[b].astype(bf16)),
                "txt": t_c,
                "msk": np.ascontiguousarray(v.reshape(njc, P).T),
                "wq": Wqb,
                "wk": Wkb,
                "wv": Wvb,
                "wo": Wob,
            }
        )
    return in_maps, nj


def kernel(img_embeds, text_embeds, text_attention_mask, Wq, Wk, Wv, Wo, bo):
    in_maps, nj = _make_in_maps(
        img_embeds, text_embeds, text_attention_mask, Wq, Wk, Wv, Wo, bo
    )
    results = run_cores(in_maps, nj)
    bo32 = np.asarray(bo, dtype=np.float32)
    return np.stack([results[b]["out"] for b in range(B)], axis=0) + bo32


def bench(in_maps, iters=10, nj: int = NJ):
    """Time repeated executions with inputs resident on device.

    Returns list of per-call seconds (dispatch + execute + sync)."""
    import time
    import jax
    import jax.numpy as jnp
    from jax.sharding import Mesh, PartitionSpec, NamedSharding

    sharded, in_names, out_names, zero_out_shapes = _get_runner(nj)[:4]
    concat_in = _concat_inputs(in_maps, in_names)
    devices = jax.devices()[:N_CORES]
    mesh = Mesh(np.asarray(devices), ("core",))
    sh = NamedSharding(mesh, PartitionSpec("core"))
    dev_in = [jax.device_put(a, sh) for a in concat_in]
    jax.block_until_ready(dev_in)

    def zeros():
        z = [
            jax.device_put(
                jnp.zeros((N_CORES * s[0],) + tuple(s[1:]), dt), sh
            )
            for (s, dt) in zero_out_shapes
        ]
        jax.block_until_ready(z)
        return z

    outs = sharded(*dev_in, *zeros())
    jax.block_until_ready(outs)
    times = []
    for _ in range(iters):
        z = zeros()
        t0 = time.perf_counter()
        outs = sharded(*dev_in, *z)
        jax.block_until_ready(outs)
        times.append(time.perf_counter() - t0)
    return times


def bench_repeat(in_maps, nj: int = NJ, repeat: int = 25, iters: int = 12):
    """Device-time via an in-NEFF For_i repeat loop: (t[repeat] - t[1]) /
    (repeat - 1). The repeat variant runs the whole main loop `repeat` times
    on device inside one dispatch, so the delta is pure device time."""
    import time
    import jax
    import jax.numpy as jnp
    from jax.sharding import Mesh, PartitionSpec, NamedSharding

    runs = {}
    for rep in (1, repeat):
        sharded, in_names, out_names, zero_out_shapes = _get_runner(nj, rep)[:4]
        concat_in = _concat_inputs(in_maps, in_names)
        devices = jax.devices()[:N_CORES]
        mesh = Mesh(np.asarray(devices), ("core",))
        sh = NamedSharding(mesh, PartitionSpec("core"))
        dev_in = [jax.device_put(a, sh) for a in concat_in]
        jax.block_until_ready(dev_in)

        def zeros(zs=zero_out_shapes, sh=sh):
            z = [
                jax.device_put(jnp.zeros((N_CORES * s[0],) + tuple(s[1:]), dt), sh)
                for (s, dt) in zs
            ]
            jax.block_until_ready(z)
            return z

        o = sharded(*dev_in, *zeros())
        jax.block_until_ready(o)
        runs[rep] = (sharded, dev_in, zeros)

    times = {1: [], repeat: []}
    for _ in range(iters):
        for rep in (1, repeat):
            sharded, dev_in, zeros = runs[rep]
            z = zeros()
            t0 = time.perf_counter()
            o = sharded(*dev_in, *z)
            jax.block_until_ready(o)
            times[rep].append(time.perf_counter() - t0)
    per = (min(times[repeat]) - min(times[1])) / (repeat - 1)
    return per, times
